# revision 1
# baseline (speedup 1.0000x reference)
"""Gated linear attention (GLA) Bass kernel for Trainium2, 8 NeuronCores.

Sharding: one core per (batch, head) pair -- B=2 x H=4 = 8 cores.
Each core computes its head's full pipeline with a chunked-parallel form of
the gated recurrence (chunk = 128), entirely on-device:

  z   = x @ (Wgk1@Wgk2)          (fused on host)
  sp  = softplus(-(z + bgk2))    = -log_sigmoid(z + bgk2)
  G   = -cumsum_per_chunk(sp)/16 (inclusive)
  qt  = (x @ Wq)^T * exp(G)*scale ; kt = (x @ Wk)^T * exp(-G)
  per chunk c:  AT = kt^T-block' qt-block  (masked s<=t)
                O  = AT^T @ V + qt^T @ S ;  S = (S + k~^T V) * exp(G_last)
  o   = O * rsqrt(mean(O^2)+eps) * (g*sigmoid(g))   [g = x @ Wg]
  out = o @ ((gnorm*Wo_head) @ Whead)               (fused on host)

Host gathers: out[b] = sum_h core_out[b,h] + bhead.
"""
import sys, os
sys.path.insert(0, "/opt/trn_rl_repo")

import numpy as np

B, T, D = 2, 2048, 512
H = 4
dk, dv = 64, 128          # per-head key/value dims
C = 128                   # chunk length
GATE_NORM = 16.0
EPS = 1e-5
SCALE = dk ** -0.5

_CACHE = {}
BF16_CHUNK = False  # bf16 chunk matmuls: 141us vs 151us but 3.5e-3 err - rejected


def build(t=T):
    import concourse.bass as bass  # noqa: F401
    from concourse import bacc, mybir
    import concourse.tile as tile
    import concourse.hw_specs as hw_specs

    F32 = mybir.dt.float32
    F32R = mybir.dt.float32r
    BF16 = mybir.dt.bfloat16
    AF = mybir.ActivationFunctionType
    OP = mybir.AluOpType
    bf = BF16_CHUNK

    # All activation funcs used here (Exp, Ln, Square, Copy, Identity) live
    # together in the natural_log_exp_and_others table, but the table chooser
    # assigns each func to the first table containing it (Exp -> exp_and_others,
    # Ln -> natural_log), which thrashes ACT_TABLE_LOADs between every Ln/Exp
    # pair (measured 41 loads, 52us).  Steer the chooser by removing our funcs
    # from every other table in the cached table dict (indices are preserved,
    # so act_func_set_id stays aligned with the compiler's act_info.json).
    need = {AF.Exp, AF.Ln, AF.Square, AF.Copy, AF.Identity}
    keep = "natural_log_exp_and_others"
    tabs = hw_specs.get_activation_tables("gen3")
    if keep in tabs and need <= tabs[keep]:
        for name, s in tabs.items():
            if name != keep:
                s -= need

    nch = t // C              # chunks
    nts = t // 512            # 512-wide time slices
    assert t % 512 == 0

    nc = bacc.Bacc("TRN2", target_bir_lowering=False, debug=False)

    xt_d = nc.dram_tensor("xt", [128, 4, t], F32R, kind="ExternalInput")
    wqk_d = nc.dram_tensor("wqk", [128, 4, 2 * dk], F32R, kind="ExternalInput")
    wvg_d = nc.dram_tensor("wvg", [128, 4, 2 * dv], F32R, kind="ExternalInput")
    wgk_d = nc.dram_tensor("wgk12", [128, 4, dk], F32R, kind="ExternalInput")
    wf_d = nc.dram_tensor("wfused", [dv, 10], F32, kind="ExternalInput")
    nb_d = nc.dram_tensor("nbgk2", [dk, 1], F32, kind="ExternalInput")
    um_d = nc.dram_tensor("umask", [C, C], F32, kind="ExternalInput")
    id_d = nc.dram_tensor("ident", [128, 128], F32, kind="ExternalInput")
    out_d = nc.dram_tensor("out10", [t, 10], F32, kind="ExternalOutput")

    with tile.TileContext(nc) as tc:
        with (
            tc.tile_pool(name="wt", bufs=1) as wt,
            tc.tile_pool(name="big", bufs=1) as big,
            tc.tile_pool(name="sm", bufs=3) as sm,
            tc.tile_pool(name="ck", bufs=5) as ck,
            tc.tile_pool(name="pp", bufs=4, space="PSUM") as pp,
            tc.tile_pool(name="pc", bufs=4, space="PSUM") as pc,
        ):
            # ---- weights / consts (small, gpsimd queue) ----
            wqk_sb = wt.tile([128, 4, 2 * dk], F32R)
            wvg_sb = wt.tile([128, 4, 2 * dv], F32R)
            wgk_sb = wt.tile([128, 4, dk], F32R)
            wf_sb = wt.tile([dv, 10], F32)
            nb_sb = wt.tile([dk, 1], F32)
            um_sb = wt.tile([C, C], F32)
            id_sb = wt.tile([128, 128], F32)
            # matmul weights on the fast sync queue ahead of the x^T stream;
            # small consts on the gpsimd queue in parallel
            nc.sync.dma_start(wgk_sb[:], wgk_d[:])
            nc.sync.dma_start(wqk_sb[:], wqk_d[:])
            nc.sync.dma_start(wvg_sb[:], wvg_d[:])
            nc.gpsimd.dma_start(wf_sb[:], wf_d[:])
            nc.gpsimd.dma_start(nb_sb[:], nb_d[:])
            nc.gpsimd.dma_start(um_sb[:], um_d[:])
            nc.gpsimd.dma_start(id_sb[:], id_d[:])
            eps_sb = wt.tile([128, 1], F32)
            nc.vector.memset(eps_sb[:], EPS)
            lnsc_sb = wt.tile([dk, 1], F32)
            nc.vector.memset(lnsc_sb[:], float(np.log(SCALE)))
            if bf:
                idb_sb = wt.tile([128, 128], BF16)
                nc.vector.tensor_copy(idb_sb[:], id_sb[:])

            # ---- big SBUF tensors ----
            xT = big.tile([128, 4, t], F32R)      # x^T per 128-d-chunk
            qt = big.tile([dk, t], F32)           # q-tilde transposed
            kt = big.tile([dk, t], F32)           # k-tilde transposed
            if bf:
                qtb = big.tile([dk, t], BF16)
                ktb = big.tile([dk, t], BF16)
            sp = big.tile([dk, t], F32)
            spc = big.tile([dk, t], F32)
            dlast = big.tile([dk, nch], F32)
            vg = big.tile([128, nch, 2 * dv], F32)               # v | g
            sw = big.tile([128, nch, dv], F32)    # g*sigmoid(g)

            spc_v = spc[:].rearrange("p (c l) -> p c l", l=C)

            # scan reset mask: 0 at chunk starts, 1 elsewhere -> one scan per
            # 512-slice does 4 independent per-chunk cumsums
            mres = wt.tile([dk, 512], F32)
            nc.vector.memset(mres[:], 1.0)
            mres_v = mres[:].rearrange("p (c l) -> p c l", l=C)
            nc.vector.memset(mres_v[:, :, 0:1], 0.0)
            ones_sb = wt.tile([dk, 1], F32)
            nc.vector.memset(ones_sb[:], 1.0)

            # ---- x^T load (HWDGE; host supplies transposed x). First slice
            # split into 128-col pieces so the first matmuls start sooner.
            for i in range(4):
                nc.sync.dma_start(xT[:, :, i * C:(i + 1) * C],
                                  xt_d[:, :, i * C:(i + 1) * C])
            for j in range(1, nts):
                nc.sync.dma_start(xT[:, :, j * 512:(j + 1) * 512],
                                  xt_d[:, :, j * 512:(j + 1) * 512])

            def emit_proj(j):
                ts = slice(j * 512, (j + 1) * 512)
                # gate chain: z -> sp = ln(1+exp(-z-b)) -> masked-reset cumsum
                pg = pp.tile([dk, 512], F32, tag="P")
                for d4 in range(4):
                    nc.tensor.matmul(pg[:], wgk_sb[:, d4, :], xT[:, d4, ts],
                                     start=(d4 == 0), stop=(d4 == 3))
                eg = sm.tile([dk, 512], F32, tag="eg")
                nc.scalar.activation(out=eg[:], in_=pg[:], func=AF.Exp,
                                     scale=-1.0, bias=nb_sb[:])
                nc.scalar.activation(out=sp[:, ts], in_=eg[:], func=AF.Ln,
                                     bias=ones_sb[:])
                nc.vector.tensor_tensor_scan(
                    out=spc[:, ts], data0=mres[:], data1=sp[:, ts],
                    initial=0.0, op0=OP.mult, op1=OP.add)
                nc.scalar.activation(
                    out=dlast[:, 4 * j:4 * j + 4],
                    in_=spc_v[:, 4 * j:4 * j + 4, C - 1:C],
                    func=AF.Exp, scale=-1.0 / GATE_NORM)
                # decay factors, stacked [q-rows | k-rows] to match pqk psum
                ee = sm.tile([128, 512], F32, tag="ee")
                nc.scalar.activation(out=ee[0:dk, :], in_=spc[:, ts], func=AF.Exp,
                                     scale=-1.0 / GATE_NORM, bias=lnsc_sb[:])
                nc.scalar.activation(out=ee[dk:2 * dk, :], in_=spc[:, ts],
                                     func=AF.Exp, scale=1.0 / GATE_NORM)

                # q|k projection (fp32r), decay applied on psum eviction
                pqk = pp.tile([128, 512], F32, tag="P")
                for d4 in range(4):
                    nc.tensor.matmul(pqk[:], wqk_sb[:, d4, :], xT[:, d4, ts],
                                     start=(d4 == 0), stop=(d4 == 3))
                nc.vector.tensor_mul(out=qt[:, ts], in0=pqk[0:dk, :],
                                     in1=ee[0:dk, :])
                nc.vector.tensor_mul(out=kt[:, ts], in0=pqk[dk:2 * dk, :],
                                     in1=ee[dk:2 * dk, :])
                if bf:
                    nc.vector.tensor_copy(out=qtb[:, ts], in_=qt[:, ts])
                    nc.vector.tensor_copy(out=ktb[:, ts], in_=kt[:, ts])

                # v|g natural projections
                for i in range(4):
                    tt = 4 * j + i
                    pn = pp.tile([128, 2 * dv], F32, tag="P")
                    for d4 in range(4):
                        nc.tensor.matmul(pn[:],
                                         xT[:, d4, tt * C:(tt + 1) * C],
                                         wvg_sb[:, d4, :],
                                         start=(d4 == 0), stop=(d4 == 3))
                    nc.vector.tensor_copy(out=vg[:, tt, :], in_=pn[:])

                # swish(g) = g * sigmoid(g) = g / (1 + exp(-g))
                gsl = vg[:, 4 * j:4 * j + 4, dv:2 * dv]
                eg2 = sm.tile([128, 4, dv], F32, tag="eg2")
                nc.scalar.activation(out=eg2[:], in_=gsl, func=AF.Exp, scale=-1.0)
                nc.vector.tensor_scalar_add(out=eg2[:], in0=eg2[:], scalar1=1.0)
                sg2 = sm.tile([128, 4, dv], F32, tag="sg2")
                nc.vector.reciprocal_approx_fast(out=sg2[:], in_=eg2[:])
                nc.vector.tensor_mul(out=sw[:, 4 * j:4 * j + 4, :],
                                     in0=sg2[:], in1=gsl)

            for j in range(nts):
                emit_proj(j)

            # ---- chunked recurrence ----
            S_prev = ck.tile([dk, dv], F32, tag="S")
            nc.vector.memset(S_prev[:], 0.0)
            for c in range(nch):
                cs = slice(c * C, (c + 1) * C)
                v_c = vg[:, c, 0:dv]
                kt_c = (ktb if bf else kt)[:, cs]
                qt_c = (qtb if bf else qt)[:, cs]

                pat = pc.tile([C, C], F32, tag="C")
                nc.tensor.matmul(pat[:], kt_c, qt_c, start=True, stop=True)
                atm = ck.tile([C, C], BF16 if bf else F32, tag="atm")
                nc.vector.tensor_mul(out=atm[:], in0=pat[:], in1=um_sb[:])

                pkt = pc.tile([C, dk], BF16 if bf else F32, tag="C")
                nc.tensor.transpose(pkt[:], kt_c,
                                    (idb_sb if bf else id_sb)[0:dk, 0:dk])
                ktn = ck.tile([C, dk], BF16 if bf else F32, tag="ktn")
                nc.scalar.copy(ktn[:], pkt[:])

                po = pc.tile([C, dv], F32, tag="C")
                nc.tensor.matmul(po[:], atm[:], v_c, start=True, stop=False)
                nc.tensor.matmul(po[:], qt[:, cs], S_prev[:],
                                 start=False, stop=True)

                pds = pc.tile([dk, dv], F32, tag="C")
                nc.tensor.matmul(pds[:], ktn[:], v_c, start=True, stop=True)
                S_new = ck.tile([dk, dv], F32, tag="S")
                nc.vector.tensor_add(out=S_new[:], in0=S_prev[:], in1=pds[:])
                nc.vector.tensor_scalar_mul(out=S_new[:], in0=S_new[:],
                                            scalar1=dlast[:, c:c + 1])
                S_prev = S_new

                # rmsnorm + gate
                scr = ck.tile([C, dv], F32, tag="scr")
                ms = ck.tile([C, 1], F32, tag="ms")
                nc.scalar.activation(out=scr[:], in_=po[:], func=AF.Square,
                                     accum_out=ms[:])
                lnv = ck.tile([C, 1], F32, tag="lnv")
                nc.scalar.activation(out=lnv[:], in_=ms[:], func=AF.Ln,
                                     scale=1.0 / dv, bias=eps_sb[:])
                rstd = ck.tile([C, 1], F32, tag="rstd")
                nc.scalar.activation(out=rstd[:], in_=lnv[:], func=AF.Exp,
                                     scale=-0.5)
                on = ck.tile([C, dv], F32, tag="on")
                nc.scalar.mul(on[:], po[:], rstd[:])
                nc.vector.tensor_mul(out=on[:], in0=on[:], in1=sw[:, c, :])

                # transpose + fused output head
                pot = pc.tile([dv, C], F32, tag="C")
                nc.tensor.transpose(pot[:], on[:], id_sb[:])
                ots = ck.tile([dv, C], F32, tag="ots")
                nc.scalar.copy(ots[:], pot[:])
                p10 = pc.tile([C, 10], F32, tag="C")
                nc.tensor.matmul(p10[:], ots[:], wf_sb[:], start=True, stop=True)
                o10 = ck.tile([C, 10], F32, tag="o10")
                nc.vector.tensor_copy(o10[:], p10[:])
                nc.sync.dma_start(out_d[cs, :], o10[:])

    nc.compile()
    return nc


def _prep_inputs(inputs, t=T):
    """Per-core input dicts: core = 4*b + h."""
    ins = {k: np.ascontiguousarray(np.asarray(v, dtype=np.float32))
           for k, v in inputs.items()}
    x, Wq, Wk, Wv, Wg = ins["x"], ins["Wq"], ins["Wk"], ins["Wv"], ins["Wg"]
    Wgk12 = (ins["Wgk1"].astype(np.float64) @ ins["Wgk2"].astype(np.float64))
    bgk2, gnorm = ins["bgk2"], ins["gnorm_w"]
    Wo, Whead = ins["Wo"], ins["Whead"]
    nch = t // C

    um = (np.arange(C)[:, None] <= np.arange(C)[None, :]).astype(np.float32)
    ident = np.eye(128, dtype=np.float32)

    def chunk_w(w):  # [512, n] -> [128, 4, n]
        return np.ascontiguousarray(w.reshape(4, 128, -1).transpose(1, 0, 2))

    in_maps = []
    for core in range(8):
        b, h = divmod(core, 4)
        wf = ((gnorm[:, None].astype(np.float64)
               * Wo[h * dv:(h + 1) * dv, :].astype(np.float64))
              @ Whead.astype(np.float64)).astype(np.float32)
        in_maps.append({
            "xt": np.ascontiguousarray(
                x[b, :t].T.reshape(4, 128, t).transpose(1, 0, 2)),
            "wqk": chunk_w(np.concatenate(
                [Wq[:, h * dk:(h + 1) * dk], Wk[:, h * dk:(h + 1) * dk]], 1)),
            "wvg": chunk_w(np.concatenate(
                [Wv[:, h * dv:(h + 1) * dv], Wg[:, h * dv:(h + 1) * dv]], 1)),
            "wgk12": chunk_w(Wgk12[:, h * dk:(h + 1) * dk].astype(np.float32)),
            "wfused": np.ascontiguousarray(wf),
            "nbgk2": np.ascontiguousarray(-bgk2[h * dk:(h + 1) * dk, None]),
            "umask": um,
            "ident": ident,
        })
    return in_maps


def _gather(results, inputs, t=T):
    bhead = np.asarray(inputs["bhead"], dtype=np.float32)
    out = np.zeros((B, t, 10), np.float32)
    for core in range(8):
        b = core // 4
        out[b] += results[core]["out10"]
    out += bhead[None, None, :]
    return out


def run(inputs, trace=False, **kw):
    from concourse.bass_utils import run_bass_kernel_spmd
    if "nc" not in _CACHE:
        _CACHE["nc"] = build()
    nc = _CACHE["nc"]
    in_maps = _prep_inputs(inputs)
    res = run_bass_kernel_spmd(nc, in_maps, core_ids=list(range(8)),
                               trace=trace, **kw)
    return _gather(res.results, inputs), res


def kernel(**inputs) -> np.ndarray:
    out, _ = run(inputs, trace=False)
    return out

